# revision 2
# baseline (speedup 1.0000x reference)
"""Multi-head self-attention kernel for Trainium2 (8 NeuronCores, Bass/Tile).

See build_core_program for the per-core dataflow.  Sharding: 8 cores =
2 batches x 4 head-groups; each core computes one batch and 4 heads end to
end (no collectives), host sums the 4 bf16 partial out-projections per batch
and folds in the v-bias correction.
"""


import numpy as np
import ml_dtypes
import sys

try:
    import concourse.bass as bass
except ImportError:  # pragma: no cover
    sys.path.insert(0, "/opt/trn_rl_repo")
    import concourse.bass as bass

import concourse.bacc as bacc
import concourse.mybir as mybir
import concourse.tile as tile
from concourse.bass_utils import run_bass_kernel_spmd

BF16 = mybir.dt.bfloat16
F32 = mybir.dt.float32
AF = mybir.ActivationFunctionType

D_MODEL = 1024
HEADS_PER_CORE = 4
HEAD_DIM = 64
CH = HEADS_PER_CORE * HEAD_DIM  # 256


def build_core_program(S=2048, D=D_MODEL, reps=1, use_gpsimd_bc=True,
                       use_fast_recip=False):
    nc = bacc.Bacc(trn_type="TRN2", target_bir_lowering=False, debug=False,
                   enable_partition_id=False)

    xT_d = nc.dram_tensor("xT", [D, S], BF16, kind="ExternalInput").ap()
    wq_d = nc.dram_tensor("wq", [D, CH], BF16, kind="ExternalInput").ap()
    wk_d = nc.dram_tensor("wk", [D, CH], BF16, kind="ExternalInput").ap()
    wv_d = nc.dram_tensor("wv", [D, CH], BF16, kind="ExternalInput").ap()
    wo_d = nc.dram_tensor("wo", [CH, D], BF16, kind="ExternalInput").ap()
    bqk_d = nc.dram_tensor("bqk", [4, 128, 1], F32, kind="ExternalInput").ap()
    out_d = nc.dram_tensor("out", [S, D], BF16, kind="ExternalOutput").ap()

    NT = S // 128     # key tiles
    ND = D // 128     # d_model contraction chunks
    NW = S // 1024    # query windows
    assert NW == 2 and NT == 16

    with tile.TileContext(nc) as tc:
        with (
            tc.tile_pool(name="persist", bufs=1) as persist,
            tc.tile_pool(name="probs", bufs=42) as probs_pool,
            tc.tile_pool(name="bc", bufs=3) as bc_pool,
            tc.tile_pool(name="outb", bufs=2) as outb_pool,
            tc.tile_pool(name="ps_sc", bufs=3, space="PSUM") as ps_sc,
            tc.tile_pool(name="ps_v", bufs=2, space="PSUM") as ps_v,
        ):
            # --- constants ------------------------------------------------
            xT = [persist.tile([128, S], BF16, name=f"xT{i}", tag=f"xT{i}")
                  for i in range(ND)]
            wq = [persist.tile([128, CH], BF16, name=f"wq{i}", tag=f"wq{i}")
                  for i in range(ND)]
            wk = [persist.tile([128, CH], BF16, name=f"wk{i}", tag=f"wk{i}")
                  for i in range(ND)]
            wv = [persist.tile([128, CH], BF16, name=f"wv{i}", tag=f"wv{i}")
                  for i in range(ND)]
            bias = [persist.tile([128, 1], F32, name=f"bias{i}", tag=f"bias{i}")
                    for i in range(4)]
            # critical path first (xT+wq+wk feed the first qk chains), split
            # across the two HWDGE trigger engines (SP + ACT)
            for i in range(ND):
                nc.scalar.dma_start(wq[i], wq_d[128 * i:128 * (i + 1), :])
            for i in range(4):
                nc.sync.dma_start(xT[i], xT_d[128 * i:128 * (i + 1), :])
            for i in range(4, ND):
                nc.scalar.dma_start(xT[i], xT_d[128 * i:128 * (i + 1), :])
            for i in range(ND):
                nc.sync.dma_start(wk[i], wk_d[128 * i:128 * (i + 1), :])
            for i in range(4):
                nc.scalar.dma_start(bias[i], bqk_d[i])
            for i in range(ND):
                nc.sync.dma_start(wv[i], wv_d[128 * i:128 * (i + 1), :])
            wo = [persist.tile([128, D], BF16, name=f"wo{i}", tag=f"wo{i}")
                  for i in range(2)]
            for i in range(2):
                nc.sync.dma_start(wo[i], wo_d[128 * i:128 * (i + 1), :])

            # dependency-free ACT warmup (loads the exp table early)
            warm = persist.tile([128, 1], F32, name="warm", tag="warm")
            nc.vector.memset(warm, 0.0)
            nc.scalar.activation(warm, warm, AF.Exp, bias=0.0, scale=1.0)

            qkT = [persist.tile([128, S], BF16, name=f"qkT{i}", tag=f"qkT{i}")
                   for i in range(4)]
            vsb = [persist.tile([128, HEADS_PER_CORE * 65], BF16,
                                name=f"v{i}", tag=f"v{i}") for i in range(NT)]
            vals = [persist.tile([128, S], BF16, name=f"vals{i}",
                                 tag=f"vals{i}") for i in range(2)]
            # 16 reciprocal rows, all on partition 0 in distinct column
            # slots (partition-offset reads/writes of 1-row APs proved
            # unreliable on HW for both DVE and gpsimd)
            rrs_t = persist.tile([1, 16 * 512], F32, name="rrs", tag="rrs")

            def rrs_row(r):
                return rrs_t[:, 512 * r:512 * (r + 1)]
            ones_col = persist.tile([1, 64], F32, name="ones_col",
                                    tag="ones_col")
            nc.vector.memset(ones_col, 1.0)

            # --- helpers --------------------------------------------------
            def qk_chain(ct, c):
                """qkT[ct][:, 512c:512c+512] = (W.T @ x) + bias."""
                wsrc = wq if ct < 2 else wk
                wcol = (ct % 2) * 128
                ps = ps_sc.tile([128, 512], F32, name="ps_qk", tag="ps_sc")
                for dc in range(ND):
                    nc.tensor.matmul(
                        ps,
                        lhsT=wsrc[dc][:, wcol:wcol + 128],
                        rhs=xT[dc][:, 512 * c:512 * (c + 1)],
                        start=(dc == 0),
                        stop=(dc == ND - 1),
                    )
                nc.vector.tensor_scalar_add(
                    qkT[ct][:, 512 * c:512 * (c + 1)], ps, bias[ct])

            def v_chain(t):
                """vsb[t]: token-major v for tile t, ones col per head.

                Uses ps_v slots: in window (0,0) no values accumulate yet,
                so both ps_v slots are free for these chains."""
                ps = ps_v.tile([128, CH], F32, name="ps_v0", tag="psv")
                for dc in range(ND):
                    nc.tensor.matmul(
                        ps,
                        lhsT=xT[dc][:, 128 * t:128 * (t + 1)],
                        rhs=wv[dc],
                        start=(dc == 0),
                        stop=(dc == ND - 1),
                    )
                nc.vector.memset(vsb[t], 1.0)
                nc.vector.tensor_copy(
                    vsb[t].rearrange("p (h c) -> p h c", c=65)[:, :, 0:64],
                    ps.rearrange("p (h c) -> p h c", c=64),
                )

            def scores_exp(p, w, j):
                """Row-packed pair scores + exp -> (probsA, probsB)."""
                psA = ps_sc.tile([128, 1024], F32, name="ps_scA", tag="ps_sc")
                psB = ps_sc.tile([128, 1024], F32, name="ps_scB", tag="ps_sc")
                for ic in range(2):
                    o = 1024 * w + 512 * ic
                    nc.tensor.matmul(
                        psA[:, 512 * ic:512 * (ic + 1)],
                        lhsT=qkT[2 + p][0:64, 128 * j:128 * (j + 1)],
                        rhs=qkT[p][0:64, o:o + 512],
                        start=True, stop=True,
                    )
                    nc.tensor.matmul(
                        psB[:, 512 * ic:512 * (ic + 1)],
                        lhsT=qkT[2 + p][64:128, 128 * j:128 * (j + 1)],
                        rhs=qkT[p][64:128, o:o + 512],
                        start=True, stop=True,
                    )
                pA = probs_pool.tile([128, 1024], BF16, name="probsA",
                                     tag="probs")
                pB = probs_pool.tile([128, 1024], BF16, name="probsB",
                                     tag="probs")
                nc.scalar.activation(pA, psA, AF.Exp, bias=0.0, scale=0.125)
                nc.scalar.activation(pB, psB, AF.Exp, bias=0.0, scale=0.125)
                return pA, pB

            def val_mm(p, hh, j, ptile, half, psV):
                h = HEADS_PER_CORE * 0 + 2 * p + hh
                nc.tensor.matmul(
                    psV,
                    lhsT=vsb[j][:, 65 * h:65 * h + 65],
                    rhs=ptile[:, 512 * half:512 * (half + 1)],
                    start=(j == 0),
                    stop=(j == NT - 1),
                )

            def drain(p, hh, w, half, psV):
                """psV [65,512] -> normalized vals quadrant."""
                r = 8 * p + 4 * w + 2 * half + hh
                if use_fast_recip:
                    nc.vector.reciprocal_approx_fast(rrs_row(r), psV[64:65, :])
                else:
                    nc.vector.reciprocal(rrs_row(r), psV[64:65, :])
                bc = bc_pool.tile([64, 512], F32, name="bc", tag="bc")
                if use_gpsimd_bc:
                    nc.gpsimd.partition_broadcast(bc, rrs_row(r))
                else:
                    psbc = ps_sc.tile([128, 512], F32, name="ps_bc",
                                      tag="ps_sc")
                    nc.tensor.matmul(psbc[0:64, :], lhsT=ones_col,
                                     rhs=rrs_row(r), start=True, stop=True)
                    nc.vector.tensor_copy(bc, psbc[0:64, :])
                qo = 1024 * w + 512 * half
                nc.vector.tensor_mul(
                    vals[p][64 * hh:64 * hh + 64, qo:qo + 512],
                    psV[0:64, :], bc)

            def outproj(t, act_ok=False):
                ob = outb_pool.tile([128, D], BF16, name="outb", tag="outb")
                for mh in range(2):
                    ps = ps_sc.tile([128, 512], F32, name="ps_out",
                                    tag="ps_sc")
                    for p in range(2):
                        nc.tensor.matmul(
                            ps,
                            lhsT=vals[p][:, 128 * t:128 * (t + 1)],
                            rhs=wo[p][:, 512 * mh:512 * (mh + 1)],
                            start=(p == 0),
                            stop=(p == 1),
                        )
                    # ACT does the copy only when exps are done (tail);
                    # mid-window it would steal from the pacing engine
                    if mh == 1 and act_ok:
                        nc.scalar.activation(ob[:, 512:1024], ps, AF.Copy,
                                             bias=0.0, scale=1.0)
                    else:
                        nc.vector.tensor_copy(
                            ob[:, 512 * mh:512 * (mh + 1)], ps)
                nc.sync.dma_start(out_d[128 * t:128 * (t + 1), :], ob)

            # --- main program --------------------------------------------
            # Window order: (p,w) = (0,0),(0,1),(1,0),(1,1).  All values
            # matmuls for window i run "pass-2 style" during window i+1
            # (half 0 in steps 0-7, half 1 in steps 8-15, two j's per step),
            # through the two ps_v slots.  Window (0,0) instead hosts the
            # v-phase chains in those slots.  qk chains for later windows are
            # spread where their psum slot + PE slack exist.
            def window_values(pp, pw, half, jjs, psVh, probs_store):
                """A few j's of values for (pp, pw, half)."""
                for jj in jjs:
                    prA, prB = probs_store[(pp, pw, jj)]
                    val_mm(pp, 0, jj, prA, half, psVh[0])
                    val_mm(pp, 1, jj, prB, half, psVh[1])

            # values j's per step: front-loaded so the previous window's
            # probs tiles release early (the probs pool is the SBUF-limited
            # resource): half 0 at steps 2-5 (4 j's each), half 1 at steps
            # 6-11 (3,3,3,3,2,2).  The 2-step lead-in keeps the next
            # window's scores from queuing behind val_mms that wait on the
            # previous drain chain to release the ps_v slots.
            VAL_SCHED = {}
            for s in range(2, 6):
                VAL_SCHED[s] = (0, list(range(4 * (s - 2), 4 * (s - 2) + 4)))
            _h1 = [3, 3, 3, 3, 2, 2]
            _off = 0
            for i, n in enumerate(_h1):
                VAL_SCHED[6 + i] = (1, list(range(_off, _off + n)))
                _off += n

            for _rep in range(reps):
                qk_chain(0, 0)
                qk_chain(0, 1)
                qk_chain(2, 0)

                probs_store = {}
                windows = [(0, 0), (0, 1), (1, 0), (1, 1)]
                for wi, (p, w) in enumerate(windows):
                    pp, pw = windows[wi - 1] if wi > 0 else (None, None)
                    psVh = None
                    for j in range(NT):
                        # late qk chains, placed just ahead of first use
                        if (p, w) == (0, 0):
                            v_chain(j)
                            if j in (1, 5, 9):
                                qk_chain(2, 1 + (j - 1) // 4)
                            elif j in (11, 13):
                                qk_chain(0, 2 + (j - 11) // 2)
                        elif (p, w) == (0, 1):
                            if j in (0, 2, 4, 6):
                                qk_chain(1, j // 2)
                            elif j in (8, 10, 12, 14):
                                qk_chain(3, (j - 8) // 2)
                        if pp is not None:
                            if j == 2:
                                psVh = [ps_v.tile([65, 512], F32, name="psVa",
                                                  tag="psv") for _ in range(2)]
                            if j == 6:
                                psVh = [ps_v.tile([65, 512], F32, name="psVb",
                                                  tag="psv") for _ in range(2)]
                        pA, pB = scores_exp(p, w, j)
                        probs_store[(p, w, j)] = (pA, pB)
                        if pp is not None and j in VAL_SCHED:
                            half, jjs = VAL_SCHED[j]
                            window_values(pp, pw, half, jjs, psVh,
                                          probs_store)
                            if j == 5:
                                drain(pp, 0, pw, 0, psVh[0])
                                drain(pp, 1, pw, 0, psVh[1])
                            elif j == 11:
                                drain(pp, 0, pw, 1, psVh[0])
                                drain(pp, 1, pw, 1, psVh[1])
                        # early out-proj: token tiles 0-3 need only query
                        # chunk 0 = half 0 of windows (p,0); pair1's drains
                        # at step 5 of this window
                        if (p, w) == (1, 1) and 8 <= j <= 14 and j % 2 == 0:
                            outproj((j - 8) // 2)

                # tail: values for the last window (both halves), with the
                # remaining out-proj tiles interleaved as their query chunks
                # drain
                for half in range(2):
                    psVh = [ps_v.tile([65, 512], F32, name="psVt", tag="psv")
                            for _ in range(2)]
                    for step in range(8):
                        window_values(1, NW - 1, half,
                                      (2 * step, 2 * step + 1), psVh,
                                      probs_store)
                    drain(1, 0, NW - 1, half, psVh[0])
                    drain(1, 1, NW - 1, half, psVh[1])
                    if half == 0:
                        for t in range(4, 8):   # chunk 1, ready since j=15
                            outproj(t, act_ok=True)
                for t in range(8, 12):          # chunk 2 = last-window half 0
                    outproj(t, act_ok=True)
                for t in range(12, NT):         # chunk 3 = last-window half 1
                    outproj(t, act_ok=True)

    nc.compile()
    return nc


def make_in_maps(x, W_qkv, b_qkv, W_out, n_cores=8):
    """Per-core input dict: core c -> batch c//4, head group c%4."""
    bf = ml_dtypes.bfloat16
    in_maps = []
    for c in range(n_cores):
        b, g = divmod(c, 4)
        heads = range(HEADS_PER_CORE * g, HEADS_PER_CORE * (g + 1))
        qs = np.concatenate([W_qkv[:, 192 * h:192 * h + 64] for h in heads], 1)
        ks = np.concatenate([W_qkv[:, 192 * h + 64:192 * h + 128] for h in heads], 1)
        vs = np.concatenate([W_qkv[:, 192 * h + 128:192 * h + 192] for h in heads], 1)
        bq = np.concatenate([b_qkv[192 * h:192 * h + 64] for h in heads])
        bk = np.concatenate([b_qkv[192 * h + 64:192 * h + 128] for h in heads])
        in_maps.append({
            "xT": np.ascontiguousarray(x[b].T).astype(bf),
            "wq": np.ascontiguousarray(qs).astype(bf),
            "wk": np.ascontiguousarray(ks).astype(bf),
            "wv": np.ascontiguousarray(vs).astype(bf),
            "wo": np.ascontiguousarray(W_out[CH * g:CH * (g + 1)]).astype(bf),
            "bqk": np.stack([bq[:128], bq[128:], bk[:128], bk[128:]])
                     .reshape(4, 128, 1).astype(np.float32),
        })
    return in_maps


_PROGRAM_CACHE = {}


def _get_program(S):
    if S not in _PROGRAM_CACHE:
        _PROGRAM_CACHE[S] = build_core_program(S=S)
    return _PROGRAM_CACHE[S]


class PjrtRunner:
    """Reusable compiled SPMD executable (no donation, so it can be re-run
    back-to-back on device-resident inputs for timing)."""

    def __init__(self, nc, n_cores=8):
        import jax
        from jax.sharding import Mesh, PartitionSpec
        from jax.experimental.shard_map import shard_map
        from concourse import bass2jax, mybir as mb

        bass2jax.install_neuronx_cc_hook()
        self.nc = nc
        self.n_cores = n_cores
        in_names, out_names, out_avals, zero_outs = [], [], [], []
        for alloc in nc.m.functions[0].allocations:
            if not isinstance(alloc, mb.MemoryLocationSet):
                continue
            name = alloc.memorylocations[0].name
            if alloc.kind == "ExternalInput":
                in_names.append(name)
            elif alloc.kind == "ExternalOutput":
                out_names.append(name)
                shape = tuple(alloc.tensor_shape)
                dtype = mb.dt.np(alloc.dtype)
                out_avals.append(jax.core.ShapedArray(shape, dtype))
                zero_outs.append(np.zeros(shape, dtype))
        self.in_names = list(in_names)
        self.out_names = out_names
        self.out_avals = out_avals
        self.zero_outs = zero_outs
        n_params = len(in_names)
        all_names = in_names + out_names

        def _body(*args):
            outs = bass2jax._bass_exec_p.bind(
                *args,
                out_avals=tuple(out_avals),
                in_names=tuple(all_names),
                out_names=tuple(out_names),
                lowering_input_output_aliases=(),
                sim_require_finite=True,
                sim_require_nnan=True,
                nc=nc,
            )
            return tuple(outs)

        devices = jax.devices()[:n_cores]
        self.mesh = Mesh(np.asarray(devices), ("core",))
        in_specs = (PartitionSpec("core"),) * (n_params + len(out_names))
        out_specs = (PartitionSpec("core"),) * len(out_names)
        self.fn = jax.jit(
            shard_map(_body, mesh=self.mesh, in_specs=in_specs,
                      out_specs=out_specs, check_rep=False),
            keep_unused=True,
        )
        self._dev_args = None

    def stage(self, in_maps):
        """Concatenate per-core inputs, upload once, keep device arrays."""
        import jax
        from jax.sharding import NamedSharding, PartitionSpec
        n = self.n_cores
        concat = [
            np.concatenate([np.asarray(in_maps[c][k]) for c in range(n)], axis=0)
            for k in self.in_names
        ]
        concat += [
            np.zeros((n * z.shape[0], *z.shape[1:]), z.dtype)
            for z in self.zero_outs
        ]
        sh = NamedSharding(self.mesh, PartitionSpec("core"))
        self._dev_args = [jax.device_put(a, sh) for a in concat]

    def run(self):
        outs = self.fn(*self._dev_args)
        # keep device arrays for reuse; pull results to host
        res = []
        for c in range(self.n_cores):
            res.append({
                name: np.asarray(outs[i]).reshape(
                    self.n_cores, *self.out_avals[i].shape)[c]
                for i, name in enumerate(self.out_names)
            })
        return res

    def time_iters(self, iters=20):
        import time
        import jax
        outs = self.fn(*self._dev_args)
        jax.block_until_ready(outs)
        t0 = time.perf_counter()
        for _ in range(iters):
            outs = self.fn(*self._dev_args)
        jax.block_until_ready(outs)
        t1 = time.perf_counter()
        return (t1 - t0) / iters


_RUNNER_CACHE = {}


def get_runner(S):
    if S not in _RUNNER_CACHE:
        _RUNNER_CACHE[S] = PjrtRunner(_get_program(S))
    return _RUNNER_CACHE[S]


def combine_outputs(results, W_qkv, b_qkv, W_out, b_out, B, S, D):
    b_v = np.concatenate([b_qkv[192 * h + 128:192 * h + 192] for h in range(16)])
    corr = (b_v.astype(np.float64) @ W_out.astype(np.float64)).astype(np.float32)
    corr += b_out
    out = np.zeros((B, S, D), np.float32)
    for c in range(8):
        out[c // 4] += results[c]["out"].astype(np.float32)
    out += corr[None, None, :]
    return out


def kernel(x, W_qkv, b_qkv, W_out, b_out):
    x = np.asarray(x)
    W_qkv = np.asarray(W_qkv)
    b_qkv = np.asarray(b_qkv)
    W_out = np.asarray(W_out)
    b_out = np.asarray(b_out)
    B, S, D = x.shape

    runner = get_runner(S)
    runner.stage(make_in_maps(x, W_qkv, b_qkv, W_out))
    results = runner.run()
    return combine_outputs(results, W_qkv, b_qkv, W_out, b_out, B, S, D)

